# revision 53
# baseline (speedup 1.0000x reference)
"""NetVLAD on 8 Trainium2 NeuronCores — self-contained kernel.

Problem: x [32, 2048, 1024] f32, W [64, 1024] f32, centroids [64, 1024] f32
  -> out [32, 65536] f32  (NetVLAD pooling: per-frame L2 norm, soft-assign
  softmax over 64 clusters, residual aggregation, intra + global L2 norm).

Sharding: data-parallel over batch — 4 samples per core, W/centroids
replicated; no cross-core communication.

Per-core design (v2): the only mandatory HBM traffic is the 33.6 MB f32
read of the x shard (~94 us at 358 GB/s), so everything else is moved off
the DMA engines:
  - x loads: SWDGE f32->bf16 cast loads, 2 MB per quarter-sample.
  - x^T for the logits matmul comes from PE transposes (identity matmul)
    instead of SBUF->SBUF xbar DMA, with PSUM->SBUF copies round-robined
    over ACT/DVE/Pool.
  - ssq = sum_d x^2 is one fused square+accumulate op per m-tile,
    round-robined over ACT (Square activation), DVE (tensor_tensor_reduce)
    and Pool (scalar_tensor_tensor).
  - softmax is batched per half-sample (8 m-tiles): one broadcast multiply
    r*z on DVE, one 512-wide EXP on ACT, one batched reduce, one broadcast
    multiply for a' = r*softmax.
  - agg/cs matmuls for half h are emitted interleaved into half h+1's PE
    stream so the PE never waits on the softmax latency.
"""

import json

import numpy as np

import concourse.bass as bass
import concourse.mybir as mybir
import concourse.tile as tile

F32 = mybir.dt.float32
BF16 = mybir.dt.bfloat16
AF = mybir.ActivationFunctionType
OP = mybir.AluOpType

B = 32
N_CORES = 8
B_PER_CORE = B // N_CORES
M = 2048
D = 1024
K = 64
NQ = 4           # quarters per sample (load granularity, 2 MB each)
TQ = 4           # m-tiles per quarter
NH = 2           # halves per sample (softmax batch granularity)
TH = 8           # m-tiles per half

# Engine round-robin for the per-tile square+accum and xT copies:
# 0 = ACT, 1 = DVE, 2 = Pool. Pool (GPSIMD) cannot read PSUM (so no xT
# copies) and its InstTensorScalarPtr hits a walrus codegen bug (so no
# fused square+accum either); Pool instead takes the SBUF-only tensor_mul
# work (a' multiply, tail muls) plus all SWDGE DMA issuing.
SSQ_PAT = [0, 1, 0, 1, 0, 1, 0, 1, 0, 1, 0, 1, 1, 0, 1, 1]  # ACT 7/16
CPY_PAT = [1, 0, 1, 1, 0, 1, 1, 1]
# Note: offloading some transposes to the DMA xbar was tried and is a net
# loss — the xbar transfers serialize against the saturated load stream
# (Tile's transpose-vs-DMA deadlock guard), so all transposes stay on PE.

_PATCHED = False


def _split_waits_json(bir: dict, max_waits: int = 1) -> dict:
    """Split multi-wait sync infos into standalone EventSemaphore waits.

    The walrus build in this image supports a single sync-wait command per
    instruction, while Tile's sem assignment emits several (e.g. the
    kernel-tail Drain waits on every DMAHW lane). Hoisting the extra waits
    into preceding single-wait EventSemaphore instructions on the same
    engine is semantics-preserving for monotonic semaphores.
    """
    ctr = 0
    for f in bir.get("functions", []):
        for blk in f.get("blocks", []):
            insts = blk.get("instructions", [])
            new = []
            for inst in insts:
                si = inst.get("sync_info")
                waits = si.get("on_wait", []) if si else []
                if len(waits) > max_waits:
                    head, keep = waits[:-max_waits], waits[-max_waits:]
                    for w in head:
                        ctr += 1
                        new.append({
                            "debug": inst.get("debug", 0),
                            "engine": inst["engine"],
                            "ins": [],
                            "name": f"{inst['name']}-wsplit{ctr}",
                            "opcode": "EventSemaphore",
                            "outs": [],
                            "sync_info": {"on_update": [], "on_wait": [w]},
                        })
                    si["on_wait"] = keep
                new.append(inst)
            blk["instructions"] = new
    return bir


def _apply_patch():
    global _PATCHED
    if _PATCHED:
        return
    import concourse.bass_utils as bu
    import concourse.bass2jax as b2j
    orig = bu.compile_bir_kernel

    def patched(bir_json, tmpdir, neff_name="file.neff"):
        d = json.loads(bir_json)
        d = _split_waits_json(d, 1)
        return orig(json.dumps(d).encode(), tmpdir, neff_name)

    bu.compile_bir_kernel = patched
    b2j.compile_bir_kernel = patched
    _PATCHED = True


def build_nc():
    nc = bass.Bass()
    x = nc.dram_tensor("x", [B_PER_CORE, M, D], F32, kind="ExternalInput")
    W = nc.dram_tensor("W", [K, D], F32, kind="ExternalInput")
    C = nc.dram_tensor("centroids", [K, D], F32, kind="ExternalInput")
    out = nc.dram_tensor("out", [B_PER_CORE, K * D], F32, kind="ExternalOutput")
    ident_d = nc.dram_tensor("ident", [128, 128], F32, kind="ExternalInput")

    xr = x[:, :, :].rearrange("s (q t p) d -> s q p t d", q=NQ, t=TQ, p=128)
    outr = out[:, :].rearrange("s (k d) -> s k d", d=D)

    from contextlib import ExitStack
    with tile.TileContext(nc) as tc, ExitStack() as es:
        singles = es.enter_context(tc.tile_pool(name="singles", bufs=1))
        xqpool = es.enter_context(tc.tile_pool(name="xqp", bufs=10))
        xTpool = es.enter_context(tc.tile_pool(name="xTp", bufs=3))
        dumpA = es.enter_context(tc.tile_pool(name="dumpA", bufs=2))
        dumpD = es.enter_context(tc.tile_pool(name="dumpD", bufs=2))
        dumpP = es.enter_context(tc.tile_pool(name="dumpP", bufs=2))
        statpool = es.enter_context(tc.tile_pool(name="statp", bufs=2))
        rzpool = es.enter_context(tc.tile_pool(name="rzp", bufs=2))
        epool = es.enter_context(tc.tile_pool(name="ep", bufs=2))
        apool = es.enter_context(tc.tile_pool(name="apl", bufs=2))
        rspool = es.enter_context(tc.tile_pool(name="rsp", bufs=4))
        tailpool = es.enter_context(tc.tile_pool(name="tailp", bufs=2))
        xTpsum = es.enter_context(tc.tile_pool(name="xTps", bufs=2, space="PSUM"))
        zpsum = es.enter_context(tc.tile_pool(name="zps", bufs=2, space="PSUM"))
        aggpsum = es.enter_context(
            tc.tile_pool(name="aggps", bufs=1, space="PSUM"))
        cspsum = es.enter_context(tc.tile_pool(name="csps", bufs=1, space="PSUM"))
        tailpsum = es.enter_context(tc.tile_pool(name="tps", bufs=1, space="PSUM"))

        # tiny prologue loads first: ident (first transpose) + W (first z)
        ident = singles.tile([128, 128], BF16)
        nc.gpsimd.dma_start(out=ident, in_=ident_d[:, :])
        Wbf = singles.tile([K, D], BF16)
        nc.gpsimd.dma_start(out=Wbf, in_=W[:, :])

        xq_tiles = {}

        def load_quarter(s, q, split=False):
            xq = xqpool.tile([128, TQ, D], BF16, tag="xq", name=f"xq_{s}_{q}")
            if split:
                # per-tile loads so the first transpose starts ~4x earlier
                for i in range(TQ):
                    nc.gpsimd.dma_start(out=xq[:, i, :], in_=xr[s, q][:, i, :])
            else:
                nc.gpsimd.dma_start(out=xq, in_=xr[s, q])
            xq_tiles[(s, q)] = xq

        def eng_of(idx):
            return [nc.scalar, nc.vector, nc.gpsimd][idx]

    # fused sum-of-squares over one [128, D] tile -> ssq column
        def ssq_op(eng_idx, xt, ssq_col, name):
            if eng_idx == 0:
                dump = dumpA.tile([128, D], BF16, tag="dA", name=f"dA_{name}")
                nc.scalar.activation(
                    out=dump, in_=xt, func=AF.Square, accum_out=ssq_col)
            else:
                dump = dumpD.tile([128, D], BF16, tag="dD", name=f"dD_{name}")
                nc.vector.scalar_tensor_tensor(
                    out=dump, in0=xt, scalar=1.0, in1=xt,
                    op0=OP.mult, op1=OP.mult, accum_out=ssq_col)

        def copy_op(eng_idx, dst, src):
            if eng_idx == 0:
                nc.scalar.copy(dst, src)
            elif eng_idx == 1:
                nc.vector.tensor_copy(out=dst, in_=src)
            else:
                nc.gpsimd.tensor_copy(out=dst, in_=src)

        def process_half(s, h, agg, cs, pending, zfifo):
            """Emit compute for half-sample (s, h).

            The previous half's stats/softmax/agg work arrives as `pending`
            closures and is drip-fed between this half's tile ops so no
            engine's program order ever blocks on the cross-engine softmax
            chain. z-matmuls are emitted 2 tiles behind their transposes
            (via `zfifo`) so the PE never stalls on the xT PSUM->SBUF copy.
            Returns this half's own closure list.
            """
            base = 64 * (s % 2)
            zb = zpsum.tile([128, TH, K], F32, tag="zb", name=f"zb_{s}_{h}")
            ssqh = statpool.tile([128, TH], F32, tag="ssqh",
                                 name=f"ssqh_{s}_{h}")
            pend = list(pending) if pending else []
            xts = []
            for i in range(TH):
                t = TH * h + i
                q, iq = t // TQ, t % TQ
                g = 16 * s + t
                xt = xq_tiles[(s, q)][:, iq, :]
                xts.append(xt)
                xT_sb = xTpool.tile([128, 8, 128], BF16, tag="xTsb",
                                    name=f"xTsb_{s}_{t}")
                xT_ps = xTpsum.tile([128, 8, 128], BF16, tag="xTps",
                                    name=f"xTps_{s}_{t}")
                for c in range(8):
                    nc.tensor.transpose(
                        xT_ps[:, c, :], xt[:, 128 * c:128 * (c + 1)], ident)

                def mkz(zslot, src):
                    def emit():
                        for c in range(8):
                            nc.tensor.matmul(
                                zslot, lhsT=src[:, c, :], rhs=WT[:, c, :],
                                start=(c == 0), stop=(c == 7))
                    return emit

                copy_op(CPY_PAT[g % len(CPY_PAT)], xT_sb, xT_ps)
                zfifo.append(mkz(zb[:, i, :], xT_sb))
                if len(zfifo) > 2:
                    zfifo.pop(0)()
                ssq_op(SSQ_PAT[g % len(SSQ_PAT)], xt, ssqh[:, i:i + 1],
                       f"{s}_{t}")
                for _ in range(3):
                    if pend:
                        pend.pop(0)()
            while pend:
                pend.pop(0)()

            # closures for this half's stats + batched softmax + agg matmuls,
            # to be emitted interleaved into the NEXT half's tile loop
            lnt = statpool.tile([128, TH], F32, tag="lnt", name=f"lnt_{s}_{h}")
            rr = statpool.tile([128, TH], F32, tag="rr", name=f"rr_{s}_{h}")
            invr = statpool.tile([128, TH], BF16, tag="invr",
                                 name=f"invr_{s}_{h}")
            rz = rzpool.tile([128, TH, K], F32, tag="rz", name=f"rz_{s}_{h}")
            e = epool.tile([128, TH, K], BF16, tag="e", name=f"e_{s}_{h}")
            sden = statpool.tile([128, TH], F32, tag="sden",
                                 name=f"sden_{s}_{h}")
            rec = statpool.tile([128, TH], F32, tag="rec", name=f"rec_{s}_{h}")
            cmul = statpool.tile([128, TH], F32, tag="cmul",
                                 name=f"cmul_{s}_{h}")
            at = apool.tile([128, TH, K], BF16, tag="at", name=f"at_{s}_{h}")

            def softmax_steps():
                yield lambda: nc.scalar.activation(
                    out=lnt, in_=ssqh, func=AF.Ln)
                yield lambda: nc.scalar.activation(
                    out=rr, in_=lnt, func=AF.Exp, scale=-0.5)
                yield lambda: nc.scalar.activation(
                    out=invr, in_=lnt, func=AF.Exp, scale=0.5)
                rr_b = rr[:, :].unsqueeze(2).broadcast_to([128, TH, K])
                yield lambda: nc.vector.tensor_mul(rz, zb[:, :, :], rr_b)
                yield lambda: nc.scalar.activation(out=e, in_=rz, func=AF.Exp)
                yield lambda: nc.vector.reduce_sum(
                    out=sden, in_=e[:, :, :], axis=mybir.AxisListType.X)
                yield lambda: nc.vector.reciprocal(rec, sden)
                yield lambda: nc.vector.tensor_mul(cmul, rec, rr)
                # on DVE, not Pool: GpSimd must stay a pure DMA-issuing
                # engine or the in-order Pool queue delays load descriptor
                # generation behind the softmax chain and starves the DMA.
                cm_b = cmul[:, :].unsqueeze(2).broadcast_to([128, TH, K])
                yield lambda: nc.vector.tensor_mul(at, e[:, :, :], cm_b)

            first = h == 0
            last = h == NH - 1

            def mk(i):
                def emit():
                    st_ = first and i == 0
                    sp_ = last and i == TH - 1
                    nc.tensor.matmul(
                        agg[base:base + 64, 0:512], lhsT=at[:, i, :],
                        rhs=xts[i][:, 0:512], start=st_, stop=sp_)
                    nc.tensor.matmul(
                        agg[base:base + 64, 512:1024], lhsT=at[:, i, :],
                        rhs=xts[i][:, 512:1024], start=st_, stop=sp_)
                    nc.tensor.matmul(
                        cs[base:base + 64, 0:1], lhsT=at[:, i, :],
                        rhs=invr[:, i:i + 1], start=st_, stop=sp_)
                return emit

            return list(softmax_steps()) + [mk(i) for i in range(TH)]

        def tail_sample(s, agg, cs):
            # intra-normalize rows of vlad, then scale by the global L2
            # factor: the 64 intra-normalized rows per sample are unit norm,
            # so the global norm is exactly sqrt(K) = 8 -> fold 1/64 into
            # the Ln input scale. Runs per sample so the even sample's tail
            # overlaps the odd sample's compute.
            b = 64 * (s % 2)
            e = b + 64
            cssb = rspool.tile([128, 1], F32, tag="cssb", name=f"cssb_{s}")
            nc.vector.tensor_copy(out=cssb[b:e, :], in_=cs[b:e, 0:1])
            tmp = tailpool.tile([128, D], F32, tag="tmp", name=f"tmp_{s}")
            nc.scalar.mul(tmp[b:e, :], cpair[b:e, :], cssb[b:e, 0:1])
            vlad = tailpool.tile([128, D], F32, tag="vlad", name=f"vlad_{s}")
            nc.vector.tensor_sub(vlad[b:e, :], agg[b:e, :], tmp[b:e, :])
            sq2 = dumpA.tile([128, D], BF16, tag="dA", name=f"sqt_{s}")
            vssq = rspool.tile([128, 1], F32, tag="vssq", name=f"vssq_{s}")
            nc.scalar.activation(out=sq2[b:e, :], in_=vlad[b:e, :],
                                 func=AF.Square, accum_out=vssq[b:e, :])
            lnv = rspool.tile([128, 1], F32, tag="lnv", name=f"lnv_{s}")
            nc.scalar.activation(out=lnv[b:e, :], in_=vssq[b:e, :],
                                 func=AF.Ln, scale=64.0)
            rv = rspool.tile([128, 1], F32, tag="rv", name=f"rv_{s}")
            nc.scalar.activation(out=rv[b:e, :], in_=lnv[b:e, :],
                                 func=AF.Exp, scale=-0.5)
            osb = tailpool.tile([128, D], F32, tag="osb", name=f"osb_{s}")
            nc.scalar.mul(osb[b:e, :], vlad[b:e, :], rv[b:e, 0:1])
            nc.sync.dma_start(out=outr[s], in_=osb[b:e, :])

        # schedule: halves in order; xq bufs=8 holds 4 halves, so loads are
        # prefetched ~3 halves ahead. Loads for half idx+3 are emitted AFTER
        # process_half(idx) — which emits the agg matmuls of half idx-1, the
        # last readers of the buffers half idx+3 will reuse — keeping the
        # pool's WAR dependencies in emission order.
        halves = [(s, h) for s in range(B_PER_CORE) for h in range(NH)]
        load_quarter(0, 0, split=True)
        load_quarter(0, 1)
        # the rest of the one-time setup after the first half's loads
        WT = singles.tile([128, 8, K], BF16)  # WT[p, c, k] = W[k, 128c+p]
        nc.sync.dma_start(out=WT, in_=Wbf, transpose=True)
        cpair = singles.tile([128, D], F32)
        nc.gpsimd.dma_start(out=cpair[0:64, :], in_=C[:, :])
        nc.gpsimd.dma_start(out=cpair[64:128, :], in_=C[:, :])
        for s, h in halves[1:4]:
            for q in (2 * h, 2 * h + 1):
                load_quarter(s, q)

        pending = None
        agg = cs = None
        zfifo = []
        for idx, (s, h) in enumerate(halves):
            p = s // 2
            if h == 0 and s % 2 == 0:
                agg = aggpsum.tile([128, D], F32, tag="agg", name=f"agg_{p}")
                cs = cspsum.tile([128, 8], F32, tag="cs", name=f"cs_{p}")
            pending = process_half(s, h, agg, cs, pending, zfifo)
            if idx + 4 < len(halves):
                s2, h2 = halves[idx + 4]
                for q in (2 * h2, 2 * h2 + 1):
                    load_quarter(s2, q)
            if h == 0 and s % 2 == 1:
                # sample s-1's agg matmuls just drained in this half
                tail_sample(s - 1, agg, cs)
            if h == NH - 1 and s % 2 == 1:
                while zfifo:
                    zfifo.pop(0)()
                for emit in pending:
                    emit()
                pending = None
                tail_sample(s, agg, cs)

    return nc


_NC_CACHE = None


def kernel(**inputs: np.ndarray) -> np.ndarray:
    global _NC_CACHE
    _apply_patch()
    from concourse.bass_utils import run_bass_kernel_spmd

    x = np.ascontiguousarray(np.asarray(inputs["x"], dtype=np.float32))
    W = np.ascontiguousarray(np.asarray(inputs["W"], dtype=np.float32))
    cent = np.ascontiguousarray(
        np.asarray(inputs["centroids"], dtype=np.float32))

    ident = np.eye(128, dtype=np.float32)

    if _NC_CACHE is None:
        _NC_CACHE = build_nc()
    nc = _NC_CACHE

    in_maps = [
        {
            "x": np.ascontiguousarray(
                x[B_PER_CORE * c:B_PER_CORE * (c + 1)]),
            "W": W,
            "centroids": cent,
            "ident": ident,
        }
        for c in range(N_CORES)
    ]
    res = run_bass_kernel_spmd(nc, in_maps, core_ids=list(range(N_CORES)))
    return np.concatenate([r["out"] for r in res.results], axis=0)


# revision 58
# speedup vs baseline: 1.1136x; 1.1136x over previous
"""NetVLAD on 8 Trainium2 NeuronCores — self-contained kernel.

Problem: x [32, 2048, 1024] f32, W [64, 1024] f32, centroids [64, 1024] f32
  -> out [32, 65536] f32  (NetVLAD pooling: per-frame L2 norm, soft-assign
  softmax over 64 clusters, residual aggregation, intra + global L2 norm).

Sharding: data-parallel over batch — 4 samples per core, W/centroids
replicated; no cross-core communication.

Per-core design (v2): the only mandatory HBM traffic is the 33.6 MB f32
read of the x shard (~94 us at 358 GB/s), so everything else is moved off
the DMA engines:
  - x loads: SWDGE f32->bf16 cast loads, 2 MB per quarter-sample.
  - x^T for the logits matmul comes from PE transposes (identity matmul)
    instead of SBUF->SBUF xbar DMA, with PSUM->SBUF copies round-robined
    over ACT/DVE/Pool.
  - ssq = sum_d x^2 is one fused square+accumulate op per m-tile,
    round-robined over ACT (Square activation), DVE (tensor_tensor_reduce)
    and Pool (scalar_tensor_tensor).
  - softmax is batched per half-sample (8 m-tiles): one broadcast multiply
    r*z on DVE, one 512-wide EXP on ACT, one batched reduce, one broadcast
    multiply for a' = r*softmax.
  - agg/cs matmuls for half h are emitted interleaved into half h+1's PE
    stream so the PE never waits on the softmax latency.
"""

import json

import numpy as np

import concourse.bass as bass
import concourse.mybir as mybir
import concourse.tile as tile

F32 = mybir.dt.float32
BF16 = mybir.dt.bfloat16
AF = mybir.ActivationFunctionType
OP = mybir.AluOpType

B = 32
N_CORES = 8
B_PER_CORE = B // N_CORES
M = 2048
D = 1024
K = 64
NQ = 4           # quarters per sample (load granularity, 2 MB each)
TQ = 4           # m-tiles per quarter
NH = 2           # halves per sample (softmax batch granularity)
TH = 8           # m-tiles per half

# Engine round-robin for the per-tile square+accum and xT copies:
# 0 = ACT, 1 = DVE, 2 = Pool. Pool (GPSIMD) cannot read PSUM (so no xT
# copies) and its InstTensorScalarPtr hits a walrus codegen bug (so no
# fused square+accum either); Pool instead takes the SBUF-only tensor_mul
# work (a' multiply, tail muls) plus all SWDGE DMA issuing.
SSQ_PAT = [0, 1, 0, 1, 0, 1, 0, 1, 0, 1, 0, 1, 1, 0, 1, 1]  # ACT 7/16
CPY_PAT = [1, 0, 1, 1, 0, 1, 1, 1]
# Note: offloading some transposes to the DMA xbar was tried and is a net
# loss — the xbar transfers serialize against the saturated load stream
# (Tile's transpose-vs-DMA deadlock guard), so all transposes stay on PE.

_PATCHED = False


def _split_waits_json(bir: dict, max_waits: int = 1) -> dict:
    """Split multi-wait sync infos into standalone EventSemaphore waits.

    The walrus build in this image supports a single sync-wait command per
    instruction, while Tile's sem assignment emits several (e.g. the
    kernel-tail Drain waits on every DMAHW lane). Hoisting the extra waits
    into preceding single-wait EventSemaphore instructions on the same
    engine is semantics-preserving for monotonic semaphores.
    """
    ctr = 0
    for f in bir.get("functions", []):
        for blk in f.get("blocks", []):
            insts = blk.get("instructions", [])
            new = []
            for inst in insts:
                si = inst.get("sync_info")
                waits = si.get("on_wait", []) if si else []
                if len(waits) > max_waits:
                    head, keep = waits[:-max_waits], waits[-max_waits:]
                    for w in head:
                        ctr += 1
                        new.append({
                            "debug": inst.get("debug", 0),
                            "engine": inst["engine"],
                            "ins": [],
                            "name": f"{inst['name']}-wsplit{ctr}",
                            "opcode": "EventSemaphore",
                            "outs": [],
                            "sync_info": {"on_update": [], "on_wait": [w]},
                        })
                    si["on_wait"] = keep
                new.append(inst)
            blk["instructions"] = new
    return bir


def _apply_patch():
    global _PATCHED
    if _PATCHED:
        return
    import concourse.bass_utils as bu
    import concourse.bass2jax as b2j
    orig = bu.compile_bir_kernel

    def patched(bir_json, tmpdir, neff_name="file.neff"):
        d = json.loads(bir_json)
        d = _split_waits_json(d, 1)
        return orig(json.dumps(d).encode(), tmpdir, neff_name)

    bu.compile_bir_kernel = patched
    b2j.compile_bir_kernel = patched
    _PATCHED = True


def build_nc():
    nc = bass.Bass()
    x = nc.dram_tensor("x", [B_PER_CORE, M, D], F32, kind="ExternalInput")
    W = nc.dram_tensor("W", [K, D], F32, kind="ExternalInput")
    C = nc.dram_tensor("centroids", [K, D], F32, kind="ExternalInput")
    out = nc.dram_tensor("out", [B_PER_CORE, K * D], F32, kind="ExternalOutput")
    ident_d = nc.dram_tensor("ident", [128, 128], F32, kind="ExternalInput")

    xr = x[:, :, :].rearrange("s (q t p) d -> s q p t d", q=NQ, t=TQ, p=128)
    outr = out[:, :].rearrange("s (k d) -> s k d", d=D)

    from contextlib import ExitStack
    with tile.TileContext(nc) as tc, ExitStack() as es:
        singles = es.enter_context(tc.tile_pool(name="singles", bufs=1))
        xqpool = es.enter_context(tc.tile_pool(name="xqp", bufs=10))
        xTpool = es.enter_context(tc.tile_pool(name="xTp", bufs=3))
        dumpA = es.enter_context(tc.tile_pool(name="dumpA", bufs=2))
        dumpD = es.enter_context(tc.tile_pool(name="dumpD", bufs=2))
        dumpP = es.enter_context(tc.tile_pool(name="dumpP", bufs=2))
        statpool = es.enter_context(tc.tile_pool(name="statp", bufs=2))
        rzpool = es.enter_context(tc.tile_pool(name="rzp", bufs=2))
        epool = es.enter_context(tc.tile_pool(name="ep", bufs=2))
        apool = es.enter_context(tc.tile_pool(name="apl", bufs=2))
        rspool = es.enter_context(tc.tile_pool(name="rsp", bufs=4))
        tailpool = es.enter_context(tc.tile_pool(name="tailp", bufs=2))
        xTlate = es.enter_context(tc.tile_pool(name="xTlate", bufs=TH))
        xTpsum = es.enter_context(tc.tile_pool(name="xTps", bufs=2, space="PSUM"))
        zpsum = es.enter_context(tc.tile_pool(name="zps", bufs=2, space="PSUM"))
        aggpsum = es.enter_context(
            tc.tile_pool(name="aggps", bufs=1, space="PSUM"))
        cspsum = es.enter_context(tc.tile_pool(name="csps", bufs=1, space="PSUM"))
        tailpsum = es.enter_context(tc.tile_pool(name="tps", bufs=1, space="PSUM"))

        # tiny prologue loads first: ident (first transpose) + W (first z)
        ident = singles.tile([128, 128], BF16)
        nc.gpsimd.dma_start(out=ident, in_=ident_d[:, :])
        Wbf = singles.tile([K, D], BF16)
        nc.gpsimd.dma_start(out=Wbf, in_=W[:, :])

        xq_tiles = {}
        late_xT = {}

        def load_quarter(s, q, split=False):
            xq = xqpool.tile([128, TQ, D], BF16, tag="xq", name=f"xq_{s}_{q}")
            if split:
                # per-tile loads so the first transpose starts ~4x earlier
                for i in range(TQ):
                    nc.gpsimd.dma_start(out=xq[:, i, :], in_=xr[s, q][:, i, :])
            else:
                nc.gpsimd.dma_start(out=xq, in_=xr[s, q])
            xq_tiles[(s, q)] = xq

        def eng_of(idx):
            return [nc.scalar, nc.vector, nc.gpsimd][idx]

    # fused sum-of-squares over one [128, D] tile -> ssq column
        def ssq_op(eng_idx, xt, ssq_col, name):
            if eng_idx == 0:
                dump = dumpA.tile([128, D], BF16, tag="dA", name=f"dA_{name}")
                nc.scalar.activation(
                    out=dump, in_=xt, func=AF.Square, accum_out=ssq_col)
            else:
                dump = dumpD.tile([128, D], BF16, tag="dD", name=f"dD_{name}")
                nc.vector.scalar_tensor_tensor(
                    out=dump, in0=xt, scalar=1.0, in1=xt,
                    op0=OP.mult, op1=OP.mult, accum_out=ssq_col)

        def copy_op(eng_idx, dst, src):
            if eng_idx == 0:
                nc.scalar.copy(dst, src)
            elif eng_idx == 1:
                nc.vector.tensor_copy(out=dst, in_=src)
            else:
                nc.gpsimd.tensor_copy(out=dst, in_=src)

        def process_half(s, h, agg, cs, pending, zfifo):
            """Emit compute for half-sample (s, h).

            The previous half's stats/softmax/agg work arrives as `pending`
            closures and is drip-fed between this half's tile ops so no
            engine's program order ever blocks on the cross-engine softmax
            chain. z-matmuls are emitted 2 tiles behind their transposes
            (via `zfifo`) so the PE never stalls on the xT PSUM->SBUF copy.
            Returns this half's own closure list.
            """
            base = 64 * (s % 2)
            zb = zpsum.tile([128, TH, K], F32, tag="zb", name=f"zb_{s}_{h}")
            ssqh = statpool.tile([128, TH], F32, tag="ssqh",
                                 name=f"ssqh_{s}_{h}")
            pend = list(pending) if pending else []
            xts = []
            for i in range(TH):
                t = TH * h + i
                q, iq = t // TQ, t % TQ
                g = 16 * s + t
                xt = xq_tiles[(s, q)][:, iq, :]
                xts.append(xt)
                if (s, t) in late_xT:
                    # last half: x^T was produced by pre-issued xbar DMA
                    # transposes running in the DMA's idle window after the
                    # final loads — skip the PE transposes and the copy.
                    xT_sb = late_xT[(s, t)]
                else:
                    xT_sb = xTpool.tile([128, 8, 128], BF16, tag="xTsb",
                                        name=f"xTsb_{s}_{t}")
                    xT_ps = xTpsum.tile([128, 8, 128], BF16, tag="xTps",
                                        name=f"xTps_{s}_{t}")
                    for c in range(8):
                        nc.tensor.transpose(
                            xT_ps[:, c, :], xt[:, 128 * c:128 * (c + 1)],
                            ident)

                def mkz(zslot, src):
                    def emit():
                        for c in range(8):
                            nc.tensor.matmul(
                                zslot, lhsT=src[:, c, :], rhs=WT[:, c, :],
                                start=(c == 0), stop=(c == 7))
                    return emit

                if (s, t) not in late_xT:
                    copy_op(CPY_PAT[g % len(CPY_PAT)], xT_sb, xT_ps)
                zfifo.append(mkz(zb[:, i, :], xT_sb))
                if len(zfifo) > 2:
                    zfifo.pop(0)()
                ssq_op(SSQ_PAT[g % len(SSQ_PAT)], xt, ssqh[:, i:i + 1],
                       f"{s}_{t}")
                for _ in range(3):
                    if pend:
                        pend.pop(0)()
            while pend:
                pend.pop(0)()

            # closures for this half's stats + batched softmax + agg matmuls,
            # to be emitted interleaved into the NEXT half's tile loop
            lnt = statpool.tile([128, TH], F32, tag="lnt", name=f"lnt_{s}_{h}")
            rr = statpool.tile([128, TH], F32, tag="rr", name=f"rr_{s}_{h}")
            invr = statpool.tile([128, TH], BF16, tag="invr",
                                 name=f"invr_{s}_{h}")
            rz = rzpool.tile([128, TH, K], F32, tag="rz", name=f"rz_{s}_{h}")
            e = epool.tile([128, TH, K], BF16, tag="e", name=f"e_{s}_{h}")
            sden = statpool.tile([128, TH], F32, tag="sden",
                                 name=f"sden_{s}_{h}")
            rec = statpool.tile([128, TH], F32, tag="rec", name=f"rec_{s}_{h}")
            cmul = statpool.tile([128, TH], F32, tag="cmul",
                                 name=f"cmul_{s}_{h}")
            at = apool.tile([128, TH, K], BF16, tag="at", name=f"at_{s}_{h}")

            def softmax_steps():
                yield lambda: nc.scalar.activation(
                    out=lnt, in_=ssqh, func=AF.Ln)
                yield lambda: nc.scalar.activation(
                    out=rr, in_=lnt, func=AF.Exp, scale=-0.5)
                yield lambda: nc.scalar.activation(
                    out=invr, in_=lnt, func=AF.Exp, scale=0.5)
                rr_b = rr[:, :].unsqueeze(2).broadcast_to([128, TH, K])
                yield lambda: nc.vector.tensor_mul(rz, zb[:, :, :], rr_b)
                yield lambda: nc.scalar.activation(out=e, in_=rz, func=AF.Exp)
                yield lambda: nc.vector.reduce_sum(
                    out=sden, in_=e[:, :, :], axis=mybir.AxisListType.X)
                yield lambda: nc.vector.reciprocal(rec, sden)
                yield lambda: nc.vector.tensor_mul(cmul, rec, rr)
                # on DVE, not Pool: GpSimd must stay a pure DMA-issuing
                # engine or the in-order Pool queue delays load descriptor
                # generation behind the softmax chain and starves the DMA.
                cm_b = cmul[:, :].unsqueeze(2).broadcast_to([128, TH, K])
                yield lambda: nc.vector.tensor_mul(at, e[:, :, :], cm_b)

            first = h == 0
            last = h == NH - 1

            def mk(i):
                def emit():
                    st_ = first and i == 0
                    sp_ = last and i == TH - 1
                    nc.tensor.matmul(
                        agg[base:base + 64, 0:512], lhsT=at[:, i, :],
                        rhs=xts[i][:, 0:512], start=st_, stop=sp_)
                    nc.tensor.matmul(
                        agg[base:base + 64, 512:1024], lhsT=at[:, i, :],
                        rhs=xts[i][:, 512:1024], start=st_, stop=sp_)
                    nc.tensor.matmul(
                        cs[base:base + 64, 0:1], lhsT=at[:, i, :],
                        rhs=invr[:, i:i + 1], start=st_, stop=sp_)
                return emit

            return list(softmax_steps()) + [mk(i) for i in range(TH)]

        def tail_sample(s, agg, cs):
            # intra-normalize rows of vlad, then scale by the global L2
            # factor: the 64 intra-normalized rows per sample are unit norm,
            # so the global norm is exactly sqrt(K) = 8 -> fold 1/64 into
            # the Ln input scale. Runs per sample so the even sample's tail
            # overlaps the odd sample's compute.
            b = 64 * (s % 2)
            e = b + 64
            cssb = rspool.tile([128, 1], F32, tag="cssb", name=f"cssb_{s}")
            nc.vector.tensor_copy(out=cssb[b:e, :], in_=cs[b:e, 0:1])
            tmp = tailpool.tile([128, D], F32, tag="tmp", name=f"tmp_{s}")
            nc.scalar.mul(tmp[b:e, :], cpair[b:e, :], cssb[b:e, 0:1])
            vlad = tailpool.tile([128, D], F32, tag="vlad", name=f"vlad_{s}")
            nc.vector.tensor_sub(vlad[b:e, :], agg[b:e, :], tmp[b:e, :])
            sq2 = dumpA.tile([128, D], BF16, tag="dA", name=f"sqt_{s}")
            vssq = rspool.tile([128, 1], F32, tag="vssq", name=f"vssq_{s}")
            nc.scalar.activation(out=sq2[b:e, :], in_=vlad[b:e, :],
                                 func=AF.Square, accum_out=vssq[b:e, :])
            lnv = rspool.tile([128, 1], F32, tag="lnv", name=f"lnv_{s}")
            nc.scalar.activation(out=lnv[b:e, :], in_=vssq[b:e, :],
                                 func=AF.Ln, scale=64.0)
            rv = rspool.tile([128, 1], F32, tag="rv", name=f"rv_{s}")
            nc.scalar.activation(out=rv[b:e, :], in_=lnv[b:e, :],
                                 func=AF.Exp, scale=-0.5)
            osb = tailpool.tile([128, D], F32, tag="osb", name=f"osb_{s}")
            nc.scalar.mul(osb[b:e, :], vlad[b:e, :], rv[b:e, 0:1])
            nc.sync.dma_start(out=outr[s], in_=osb[b:e, :])

        # schedule: halves in order; xq bufs=8 holds 4 halves, so loads are
        # prefetched ~3 halves ahead. Loads for half idx+3 are emitted AFTER
        # process_half(idx) — which emits the agg matmuls of half idx-1, the
        # last readers of the buffers half idx+3 will reuse — keeping the
        # pool's WAR dependencies in emission order.
        halves = [(s, h) for s in range(B_PER_CORE) for h in range(NH)]
        load_quarter(0, 0, split=True)
        load_quarter(0, 1)
        # the rest of the one-time setup after the first half's loads
        WT = singles.tile([128, 8, K], BF16)  # WT[p, c, k] = W[k, 128c+p]
        nc.sync.dma_start(out=WT, in_=Wbf, transpose=True)
        cpair = singles.tile([128, D], F32)
        nc.gpsimd.dma_start(out=cpair[0:64, :], in_=C[:, :])
        nc.gpsimd.dma_start(out=cpair[64:128, :], in_=C[:, :])
        for s, h in halves[1:4]:
            for q in (2 * h, 2 * h + 1):
                load_quarter(s, q)

        pending = None
        agg = cs = None
        zfifo = []
        for idx, (s, h) in enumerate(halves):
            p = s // 2
            if h == 0 and s % 2 == 0:
                agg = aggpsum.tile([128, D], F32, tag="agg", name=f"agg_{p}")
                cs = cspsum.tile([128, 8], F32, tag="cs", name=f"cs_{p}")
            pending = process_half(s, h, agg, cs, pending, zfifo)
            if idx + 4 < len(halves):
                s2, h2 = halves[idx + 4]
                for q in (2 * h2, 2 * h2 + 1):
                    load_quarter(s2, q)
                if idx + 4 == len(halves) - 1:
                    # last half: pre-issue its 8 xbar transposes now — they
                    # are last in the DMA stream, executing after the final
                    # loads when the SDMA engines are otherwise idle.
                    for i2 in range(TH):
                        t2 = TH * h2 + i2
                        q2, iq2 = t2 // TQ, t2 % TQ
                        lt = xTlate.tile([128, 8, 128], BF16, tag="xtl",
                                         name=f"xtl_{s2}_{t2}")
                        nc.sync.dma_start(
                            out=lt, in_=xq_tiles[(s2, q2)][:, iq2, :],
                            transpose=True)
                        late_xT[(s2, t2)] = lt
            if h == 0 and s % 2 == 1:
                # sample s-1's agg matmuls just drained in this half
                tail_sample(s - 1, agg, cs)
            if h == NH - 1 and s % 2 == 1:
                while zfifo:
                    zfifo.pop(0)()
                for emit in pending:
                    emit()
                pending = None
                tail_sample(s, agg, cs)

    return nc


_NC_CACHE = None


def kernel(**inputs: np.ndarray) -> np.ndarray:
    global _NC_CACHE
    _apply_patch()
    from concourse.bass_utils import run_bass_kernel_spmd

    x = np.ascontiguousarray(np.asarray(inputs["x"], dtype=np.float32))
    W = np.ascontiguousarray(np.asarray(inputs["W"], dtype=np.float32))
    cent = np.ascontiguousarray(
        np.asarray(inputs["centroids"], dtype=np.float32))

    ident = np.eye(128, dtype=np.float32)

    if _NC_CACHE is None:
        _NC_CACHE = build_nc()
    nc = _NC_CACHE

    in_maps = [
        {
            "x": np.ascontiguousarray(
                x[B_PER_CORE * c:B_PER_CORE * (c + 1)]),
            "W": W,
            "centroids": cent,
            "ident": ident,
        }
        for c in range(N_CORES)
    ]
    res = run_bass_kernel_spmd(nc, in_maps, core_ids=list(range(N_CORES)))
    return np.concatenate([r["out"] for r in res.results], axis=0)
